# revision 6
# baseline (speedup 1.0000x reference)
"""Trainium2 Bass kernel for CoevolutionDeletionHead (8-core SPMD).

Contract: kernel(**inputs) takes the FULL unsharded inputs (numpy, keyed as in
setup_inputs) and returns the FULL outputs (logits, eij, ei).

Sharding: the i dimension (N_res=384 rows of zij / eij / output) is split into
8 slabs of 48 rows, one per NeuronCore. Each core receives its zij row-slab
and column-slab pre-transposed on the host to [i, d, j] layout so the dp=128
contraction dim lands on SBUF partitions (no on-device transposes). msa and
the small weights are replicated.

Device math (exact algebraic refactor of the reference):
  s = z[i,:,j] + z[j,:,i]            (= 2*zs, [d=128, j] tiles)
  LN(zs) @ Wp + bp  ==  (s@Wg - mean(s)*G) * rsqrt(var(s) + 4*eps) + c0
     with Wg = lnp_g*Wp, G = lnp_g@Wp, c0 = lnp_b@Wp + bp
  (the 0.5 symmetrization scale cancels exactly when eps is scaled by 4)
  row stats come from the same matmul: rhs = [Wg | ones] gives sum(s);
  a second 1-column matmul on s^2 gives sum(s^2).
  hi[i,m] = sum_{c<21, j} eij[i,j,c] * (msa[m,j]==c)  as 63 accumulating
  matmuls (per-class one-hot built on-chip by is_equal against msa^T).
  ei head: gelu_exact via Erf, LN folded into the final dot product.
"""

import numpy as np

# problem dims (hardcoded per contract)
B, N, NSEQ, DS, DP, C = 1, 384, 256, 384, 128, 22
GAP = 21
NCORES = 8
SLAB = N // NCORES          # 48
JT = N // 128               # 3 j-tiles
EPS = 1e-5
F32 = np.float32

_CACHE = {}


def _build_program(repeat=1, act="erf"):
    import concourse.bacc as bacc
    import concourse.bass as bass
    import concourse.mybir as mybir
    import concourse.tile as tile

    f32 = mybir.dt.float32
    nc = bacc.Bacc("TRN2", target_bir_lowering=False, debug=False)

    # ---- dram tensors (per-core inputs) ----
    zrT = nc.dram_tensor("zrT", [SLAB, DP, N], f32, kind="ExternalInput")
    zcT = nc.dram_tensor("zcT", [SLAB, DP, N], f32, kind="ExternalInput")
    msaT = nc.dram_tensor("msaT", [N, NSEQ], f32, kind="ExternalInput")
    siT = nc.dram_tensor("siT", [DS, SLAB], f32, kind="ExternalInput")
    W1 = nc.dram_tensor("W1", [DS, DS], f32, kind="ExternalInput")
    wext = nc.dram_tensor("wext", [DP, C + 1], f32, kind="ExternalInput")
    gvec = nc.dram_tensor("gvec", [1, C], f32, kind="ExternalInput")
    c0p = nc.dram_tensor("c0p", [1, C], f32, kind="ExternalInput")
    gw1 = nc.dram_tensor("gw1", [1, DS], f32, kind="ExternalInput")
    b1r = nc.dram_tensor("b1r", [1, DS], f32, kind="ExternalInput")
    consts = nc.dram_tensor("consts", [1, 2], f32, kind="ExternalInput")
    maskt = nc.dram_tensor("maskt", [128, JT * SLAB], f32, kind="ExternalInput")

    out_eij = nc.dram_tensor("out_eij", [SLAB, N, C], f32, kind="ExternalOutput")
    out_hi = nc.dram_tensor("out_hi", [SLAB, NSEQ], f32, kind="ExternalOutput")
    out_ei = nc.dram_tensor("out_ei", [SLAB, 1], f32, kind="ExternalOutput")

    def bcast(ap, p):
        # DMA-source AP replicating a [1, n] dram row across p partitions
        return bass.AP(tensor=ap.tensor, offset=ap.offset,
                       ap=[[0, p]] + list(ap.ap[1:]))

    AX = mybir.AxisListType
    OP = mybir.AluOpType
    ACT_F = mybir.ActivationFunctionType

    with tile.TileContext(nc) as tc:
        with (
            tc.tile_pool(name="const", bufs=1) as constp,
            tc.tile_pool(name="zbig", bufs=2) as zbig,
            tc.tile_pool(name="small", bufs=4) as small,
            tc.tile_pool(name="epool", bufs=2) as epool,
            tc.tile_pool(name="ohp", bufs=4) as ohp,
            tc.tile_pool(name="stg", bufs=4) as stg,
            tc.tile_pool(name="ps12", bufs=2, space="PSUM") as ps12,
            tc.tile_pool(name="pshi", bufs=1, space="PSUM") as pshi,
            tc.tile_pool(name="psei", bufs=1, space="PSUM") as psei,
        ):
            # ---------------- constants (outside repeat loop) ----------------
            msa_sb = [constp.tile([128, NSEQ], f32, tag=f"msa{j}", name=f"msa_sb{j}") for j in range(JT)]
            for j in range(JT):
                nc.sync.dma_start(out=msa_sb[j], in_=msaT[j * 128:(j + 1) * 128, :])
            wext_sb = constp.tile([DP, C + 1], f32, tag="wext")
            nc.sync.dma_start(out=wext_sb, in_=wext[:, :])
            mask_sb = constp.tile([128, JT * SLAB], f32, tag="mask")
            nc.sync.dma_start(out=mask_sb, in_=maskt[:, :])
            gvec_sb = constp.tile([128, C], f32, tag="gvec")
            nc.sync.dma_start(out=gvec_sb, in_=bcast(gvec[:, :], 128))
            c0p_sb = constp.tile([128, C], f32, tag="c0p")
            nc.sync.dma_start(out=c0p_sb, in_=bcast(c0p[:, :], 128))
            gw1_sb = constp.tile([SLAB, DS], f32, tag="gw1")
            nc.sync.dma_start(out=gw1_sb, in_=bcast(gw1[:, :], SLAB))
            b1r_sb = constp.tile([SLAB, DS], f32, tag="b1r")
            nc.sync.dma_start(out=b1r_sb, in_=bcast(b1r[:, :], SLAB))
            consts_sb = constp.tile([SLAB, 2], f32, tag="consts")
            nc.sync.dma_start(out=consts_sb, in_=bcast(consts[:, :], SLAB))
            siT_sb = [constp.tile([128, SLAB], f32, tag=f"siT{k}", name=f"siT_sb{k}") for k in range(3)]
            W1_sb = [constp.tile([128, DS], f32, tag=f"W1{k}", name=f"W1_sb{k}") for k in range(3)]
            for k in range(3):
                nc.sync.dma_start(out=siT_sb[k], in_=siT[k * 128:(k + 1) * 128, :])
                nc.sync.dma_start(out=W1_sb[k], in_=W1[k * 128:(k + 1) * 128, :])
            ones_col = constp.tile([128, 1], f32, tag="ones")
            nc.vector.memset(ones_col, 1.0)
            eps4_col = constp.tile([128, 1], f32, tag="eps4")
            nc.vector.memset(eps4_col, 4.0 * EPS)
            eps1_col = constp.tile([SLAB, 1], f32, tag="eps1")
            nc.vector.memset(eps1_col, EPS)
            ei_col = constp.tile([SLAB, 1], f32, tag="eicol")

            import contextlib
            loop_cm = tc.For_i(0, repeat, 1) if repeat > 1 else contextlib.nullcontext()
            with loop_cm:
                # ---------------- ei head ----------------
                ps_e = psei.tile([SLAB, DS], f32)
                for k in range(3):
                    nc.tensor.matmul(ps_e, siT_sb[k], W1_sb[k],
                                     start=(k == 0), stop=(k == 2))
                h_sb = small.tile([SLAB, DS], f32, tag="h")
                nc.vector.tensor_add(out=h_sb, in0=ps_e, in1=b1r_sb)
                e_sb = small.tile([SLAB, DS], f32, tag="erf")
                act_fn = ACT_F.Erf if act == "erf" else ACT_F.Tanh
                nc.scalar.activation(out=e_sb, in_=h_sb, func=act_fn,
                                     scale=0.7071067811865476)
                nc.vector.tensor_scalar_add(out=e_sb, in0=e_sb, scalar1=1.0)
                hg_sb = small.tile([SLAB, DS], f32, tag="hg")
                nc.vector.scalar_tensor_tensor(out=hg_sb, in0=h_sb, scalar=0.5,
                                               in1=e_sb, op0=OP.mult, op1=OP.mult)
                stats = small.tile([SLAB, 6], f32, tag="bnst")
                nc.vector.bn_stats(out=stats, in_=hg_sb)
                mv = small.tile([SLAB, 2], f32, tag="bnagg")
                nc.vector.bn_aggr(out=mv, in_=stats)
                tmp = small.tile([SLAB, DS], f32, tag="eitmp")
                nc.vector.tensor_mul(out=tmp, in0=hg_sb, in1=gw1_sb)
                s_col = small.tile([SLAB, 1], f32, tag="scol")
                nc.vector.tensor_reduce(out=s_col, in_=tmp, axis=AX.X, op=OP.add)
                std_e = small.tile([SLAB, 1], f32, tag="stde")
                nc.scalar.activation(out=std_e, in_=mv[:, 1:2], func=ACT_F.Sqrt,
                                     bias=eps1_col, scale=1.0)
                rstd_e = small.tile([SLAB, 1], f32, tag="rstde")
                nc.vector.reciprocal(out=rstd_e, in_=std_e)
                t1 = small.tile([SLAB, 1], f32, tag="t1")
                nc.vector.tensor_mul(out=t1, in0=mv[:, 0:1], in1=consts_sb[:, 0:1])
                t2 = small.tile([SLAB, 1], f32, tag="t2")
                nc.vector.tensor_sub(out=t2, in0=s_col, in1=t1)
                t3 = small.tile([SLAB, 1], f32, tag="t3")
                nc.vector.tensor_mul(out=t3, in0=t2, in1=rstd_e)
                nc.vector.tensor_add(out=ei_col, in0=t3, in1=consts_sb[:, 1:2])
                nc.sync.dma_start(out=out_ei[:, :], in_=ei_col)

                # ---------------- pair head + einsum ----------------
                ps_hi = [pshi.tile([SLAB, NSEQ], f32, tag=f"hi{j}", name=f"ps_hi{j}") for j in range(JT)]
                for jt in range(JT):
                    jsl = slice(jt * 128, (jt + 1) * 128)
                    zr_big = zbig.tile([128, SLAB, 128], f32, tag="zr")
                    zc_big = zbig.tile([128, SLAB, 128], f32, tag="zc")
                    nc.sync.dma_start(
                        out=zr_big, in_=zrT[:, :, jsl].rearrange("i d j -> d i j"))
                    nc.sync.dma_start(
                        out=zc_big, in_=zcT[:, :, jsl].rearrange("i d j -> d i j"))
                    # s = zr + zc  (== 2*zs), in place into zr_big
                    nc.vector.tensor_add(out=zr_big, in0=zr_big, in1=zc_big)

                    E_jt = epool.tile([128, C, SLAB], f32, tag="E")
                    for i in range(SLAB):
                        s_t = zr_big[:, i, :]
                        s2_t = small.tile([128, 128], f32, tag="s2")
                        nc.scalar.activation(out=s2_t, in_=s_t, func=ACT_F.Square)
                        p12 = ps12.tile([128, C + 2], f32, tag="p12")
                        nc.tensor.matmul(p12[:, 0:C + 1], s_t, wext_sb,
                                         start=True, stop=True)
                        nc.tensor.matmul(p12[:, C + 1:C + 2], s2_t, ones_col,
                                         start=True, stop=True)
                        m_t = small.tile([128, 1], f32, tag="mt")
                        nc.vector.tensor_scalar_mul(out=m_t, in0=p12[:, C:C + 1],
                                                    scalar1=1.0 / DP)
                        sqmu = small.tile([128, 1], f32, tag="sqmu")
                        nc.vector.tensor_mul(out=sqmu, in0=m_t, in1=m_t)
                        v_t = small.tile([128, 1], f32, tag="vt")
                        nc.vector.tensor_scalar(out=v_t, in0=p12[:, C + 1:C + 2],
                                                scalar1=1.0 / DP, scalar2=sqmu,
                                                op0=OP.mult, op1=OP.subtract)
                        std_t = small.tile([128, 1], f32, tag="stdt")
                        nc.scalar.activation(out=std_t, in_=v_t, func=ACT_F.Sqrt,
                                             bias=eps4_col, scale=1.0)
                        r_t = small.tile([128, 1], f32, tag="rt")
                        nc.vector.reciprocal(out=r_t, in_=std_t)
                        t_mg = small.tile([128, C], f32, tag="tmg")
                        nc.vector.scalar_tensor_tensor(
                            out=t_mg, in0=gvec_sb, scalar=m_t, in1=p12[:, 0:C],
                            op0=OP.mult, op1=OP.subtract)
                        eij_w = small.tile([128, C], f32, tag="eijw")
                        nc.vector.scalar_tensor_tensor(
                            out=eij_w, in0=t_mg, scalar=r_t, in1=c0p_sb,
                            op0=OP.mult, op1=OP.subtract)
                        # eij_w == -eij ; mask columns hold {-1, 0}
                        mcol = mask_sb[:, jt * SLAB + i: jt * SLAB + i + 1]
                        nc.vector.tensor_scalar_mul(out=E_jt[:, :, i], in0=eij_w,
                                                    scalar1=mcol)
                        stage_t = stg.tile([128, C], f32, tag="stage")
                        nc.vector.tensor_scalar_mul(out=stage_t, in0=eij_w,
                                                    scalar1=mcol)
                        nc.sync.dma_start(out=out_eij[i, jsl, :], in_=stage_t)

                    # einsum over this j-block: 21 non-gap classes
                    for c in range(C - 1):
                        oh_t = ohp.tile([128, NSEQ], f32, tag="oh")
                        nc.vector.tensor_single_scalar(
                            out=oh_t, in_=msa_sb[jt], scalar=float(c),
                            op=OP.is_equal)
                        nc.tensor.matmul(ps_hi[jt], E_jt[:, c, :], oh_t,
                                         start=(c == 0), stop=(c == C - 2))

                a_t = small.tile([SLAB, NSEQ], f32, tag="hiacc")
                nc.vector.tensor_scalar(out=a_t, in0=ps_hi[0], scalar1=ei_col,
                                        scalar2=None, op0=OP.add)
                b_t = small.tile([SLAB, NSEQ], f32, tag="hiacc2")
                nc.vector.tensor_add(out=b_t, in0=ps_hi[1], in1=a_t)
                hi_sb = small.tile([SLAB, NSEQ], f32, tag="hisb")
                nc.vector.tensor_add(out=hi_sb, in0=ps_hi[2], in1=b_t)
                nc.sync.dma_start(out=out_hi[:, :], in_=hi_sb)

    nc.compile()
    return nc


def _build_jitted(nc, n_cores):
    """Reusable jitted SPMD executor (mirrors bass2jax.run_bass_via_pjrt)."""
    import jax
    import concourse.mybir as mybir
    from jax.sharding import Mesh, PartitionSpec
    from jax.experimental.shard_map import shard_map
    from concourse.bass2jax import (_bass_exec_p, install_neuronx_cc_hook,
                                    partition_id_tensor)

    install_neuronx_cc_hook()
    partition_name = nc.partition_id_tensor.name if nc.partition_id_tensor else None
    in_names, out_names, out_avals, zero_outs = [], [], [], []
    for alloc in nc.m.functions[0].allocations:
        if not isinstance(alloc, mybir.MemoryLocationSet):
            continue
        name = alloc.memorylocations[0].name
        if alloc.kind == "ExternalInput":
            if name != partition_name:
                in_names.append(name)
        elif alloc.kind == "ExternalOutput":
            out_names.append(name)
            shape = tuple(alloc.tensor_shape)
            dtype = mybir.dt.np(alloc.dtype)
            out_avals.append(jax.core.ShapedArray(shape, dtype))
            zero_outs.append(np.zeros(shape, dtype))
    n_params = len(in_names)
    full_in_names = list(in_names) + list(out_names)
    if partition_name is not None:
        full_in_names.append(partition_name)

    def _body(*args):
        operands = list(args)
        if partition_name is not None:
            operands.append(partition_id_tensor())
        outs = _bass_exec_p.bind(
            *operands,
            out_avals=tuple(out_avals),
            in_names=tuple(full_in_names),
            out_names=tuple(out_names),
            lowering_input_output_aliases=(),
            sim_require_finite=True,
            sim_require_nnan=True,
            nc=nc,
        )
        return tuple(outs)

    devices = jax.devices()[:n_cores]
    mesh = Mesh(np.asarray(devices), ("core",))
    in_specs = (PartitionSpec("core"),) * (n_params + len(out_names))
    out_specs = (PartitionSpec("core"),) * len(out_names)
    sharded = jax.jit(
        shard_map(_body, mesh=mesh, in_specs=in_specs, out_specs=out_specs,
                  check_rep=False),
        keep_unused=True,
    )
    return sharded, mesh, in_names, out_names, zero_outs


def get_executor(repeat=1):
    key = ("exec", repeat)
    if key not in _CACHE:
        nc = _build_program(repeat=repeat)
        _CACHE[key] = _build_jitted(nc, NCORES)
    return _CACHE[key]


def prepare_in_maps(si, zij, msa, W1, b1, ln1_g, ln1_b, W2, b2,
                    lnp_g, lnp_b, Wp, bp):
    si = np.asarray(si, F32)
    zij = np.asarray(zij, F32)
    msa = np.asarray(msa)
    W1 = np.asarray(W1, F32); b1 = np.asarray(b1, F32)
    ln1_g = np.asarray(ln1_g, F32); ln1_b = np.asarray(ln1_b, F32)
    W2 = np.asarray(W2, F32); b2 = np.asarray(b2, F32)
    lnp_g = np.asarray(lnp_g, F32); lnp_b = np.asarray(lnp_b, F32)
    Wp = np.asarray(Wp, F32); bp = np.asarray(bp, F32)

    z = zij[0]
    msaT = np.ascontiguousarray(msa[0].T.astype(F32))            # [N, NSEQ]
    Wg = lnp_g[:, None] * Wp                                     # [DP, C]
    wext = np.ascontiguousarray(
        np.concatenate([Wg, np.ones((DP, 1), F32)], axis=1))     # [DP, C+1]
    gvec = np.ascontiguousarray((lnp_g @ Wp)[None])              # [1, C]
    c0p = np.ascontiguousarray((lnp_b @ Wp + bp)[None])          # [1, C]
    gw1 = np.ascontiguousarray((ln1_g * W2[:, 0])[None])         # [1, DS]
    consts = np.array([[gw1.sum(), float(ln1_b @ W2[:, 0] + b2[0])]], F32)
    b1row = np.ascontiguousarray(b1[None])                       # [1, DS]
    W1c = np.ascontiguousarray(W1)

    in_maps = []
    for d in range(NCORES):
        i0 = d * SLAB
        zr = np.ascontiguousarray(z[i0:i0 + SLAB].transpose(0, 2, 1))
        zc = np.ascontiguousarray(z[:, i0:i0 + SLAB].transpose(1, 2, 0))
        siT = np.ascontiguousarray(si[0, i0:i0 + SLAB].T)
        m = np.full((128, JT * SLAB), -1.0, F32)
        for i in range(SLAB):
            g = i0 + i
            m[g % 128, (g // 128) * SLAB + i] = 0.0
        in_maps.append({
            "zrT": zr, "zcT": zc, "msaT": msaT, "siT": siT, "W1": W1c,
            "wext": wext, "gvec": gvec, "c0p": c0p, "gw1": gw1,
            "b1r": b1row, "consts": consts, "maskt": m,
        })
    return in_maps


def run_spmd(in_maps, repeat=1, device_inputs=None, convert=True):
    """Execute; returns (per_core_results, timing_closure_inputs)."""
    import jax
    from jax.sharding import NamedSharding, PartitionSpec

    sharded, mesh, in_names, out_names, zero_outs = get_executor(repeat)
    if device_inputs is None:
        sh = NamedSharding(mesh, PartitionSpec("core"))
        concat_in = [np.concatenate([m[n] for m in in_maps], axis=0)
                     for n in in_names]
        concat_zero = [np.zeros((NCORES * z.shape[0], *z.shape[1:]), z.dtype)
                       for z in zero_outs]
        device_inputs = ([jax.device_put(a, sh) for a in concat_in],
                         [jax.device_put(a, sh) for a in concat_zero])
    dev_in, dev_zero = device_inputs
    outs = sharded(*dev_in, *dev_zero)
    jax.block_until_ready(outs)
    if not convert:
        return None, device_inputs
    per_core = []
    for c in range(NCORES):
        r = {}
        for idx, n in enumerate(out_names):
            full = np.asarray(outs[idx])
            shp = zero_outs[idx].shape
            r[n] = full.reshape(NCORES, *shp)[c]
        per_core.append(r)
    return per_core, device_inputs


def kernel(si, zij, msa, W1, b1, ln1_g, ln1_b, W2, b2, lnp_g, lnp_b, Wp, bp):
    in_maps = prepare_in_maps(si, zij, msa, W1, b1, ln1_g, ln1_b, W2, b2,
                              lnp_g, lnp_b, Wp, bp)
    results, _ = run_spmd(in_maps, repeat=1)

    logits = np.zeros((B, NSEQ, N, 1), F32)
    eij = np.zeros((B, N, N, C), F32)
    ei = np.zeros((B, N, 1), F32)
    for d in range(NCORES):
        i0 = d * SLAB
        r = results[d]
        eij[0, i0:i0 + SLAB] = r["out_eij"]
        logits[0, :, i0:i0 + SLAB, 0] = r["out_hi"].T
        ei[0, i0:i0 + SLAB, 0] = r["out_ei"][:, 0]
    return logits, eij, ei


# revision 8
# speedup vs baseline: 1.1566x; 1.1566x over previous
"""Trainium2 Bass kernel for CoevolutionDeletionHead (8-core SPMD).

Contract: kernel(**inputs) takes the FULL unsharded inputs (numpy, keyed as in
setup_inputs) and returns the FULL outputs (logits, eij, ei).

Sharding: the i dimension (N_res=384 rows of zij / eij / output) is split into
8 slabs of 48 rows, one per NeuronCore. Each core receives its zij row-slab
and column-slab pre-transposed on the host to [i, d, j] layout so the dp=128
contraction dim lands on SBUF partitions (no on-device transposes). msa and
the small weights are replicated.

Device math (exact algebraic refactor of the reference):
  s = z[i,:,j] + z[j,:,i]            (= 2*zs, [d=128, j] tiles)
  LN(zs) @ Wp + bp  ==  (s@Wg - mean(s)*G) * rsqrt(var(s) + 4*eps) + c0
     with Wg = lnp_g*Wp, G = lnp_g@Wp, c0 = lnp_b@Wp + bp
  (the 0.5 symmetrization scale cancels exactly when eps is scaled by 4)
  row stats come from the same matmul: rhs = [Wg | ones] gives sum(s);
  a second 1-column matmul on s^2 gives sum(s^2).
  hi[i,m] = sum_{c<21, j} eij[i,j,c] * (msa[m,j]==c)  as 63 accumulating
  matmuls (per-class one-hot built on-chip by is_equal against msa^T).
  ei head: gelu_exact via Erf, LN folded into the final dot product.
"""

import numpy as np

# problem dims (hardcoded per contract)
B, N, NSEQ, DS, DP, C = 1, 384, 256, 384, 128, 22
GAP = 21
NCORES = 8
SLAB = N // NCORES          # 48
JT = N // 128               # 3 j-tiles
EPS = 1e-5
F32 = np.float32

_CACHE = {}


def _build_program(repeat=1, act="erf", einsum_f32r=True):
    import concourse.bacc as bacc
    import concourse.bass as bass
    import concourse.mybir as mybir
    import concourse.tile as tile

    f32 = mybir.dt.float32
    f32r = mybir.dt.float32r
    nc = bacc.Bacc("TRN2", target_bir_lowering=False, debug=False)

    # ---- dram tensors (per-core inputs) ----
    zrT = nc.dram_tensor("zrT", [SLAB, DP, N], f32, kind="ExternalInput")
    zcT = nc.dram_tensor("zcT", [SLAB, DP, N], f32, kind="ExternalInput")
    msaT = nc.dram_tensor("msaT", [N, NSEQ], f32, kind="ExternalInput")
    siT = nc.dram_tensor("siT", [DS, SLAB], f32, kind="ExternalInput")
    W1 = nc.dram_tensor("W1", [DS, DS], f32, kind="ExternalInput")
    wext = nc.dram_tensor("wext", [DP, C + 1], f32, kind="ExternalInput")
    gvec = nc.dram_tensor("gvec", [1, C], f32, kind="ExternalInput")
    c0p = nc.dram_tensor("c0p", [1, C], f32, kind="ExternalInput")
    gw1 = nc.dram_tensor("gw1", [1, DS], f32, kind="ExternalInput")
    b1r = nc.dram_tensor("b1r", [1, DS], f32, kind="ExternalInput")
    consts = nc.dram_tensor("consts", [1, 2], f32, kind="ExternalInput")
    maskt = nc.dram_tensor("maskt", [128, JT * SLAB], f32, kind="ExternalInput")

    out_eij = nc.dram_tensor("out_eij", [SLAB, N, C], f32, kind="ExternalOutput")
    out_hi = nc.dram_tensor("out_hi", [SLAB, NSEQ], f32, kind="ExternalOutput")
    out_ei = nc.dram_tensor("out_ei", [SLAB, 1], f32, kind="ExternalOutput")

    def bcast(ap, p):
        # DMA-source AP replicating a [1, n] dram row across p partitions
        return bass.AP(tensor=ap.tensor, offset=ap.offset,
                       ap=[[0, p]] + list(ap.ap[1:]))

    AX = mybir.AxisListType
    OP = mybir.AluOpType
    ACT_F = mybir.ActivationFunctionType
    G16 = 16                      # i-group size for batched stats
    NG = SLAB // G16              # 3 groups per jt

    edt = f32r if einsum_f32r else f32

    with tile.TileContext(nc) as tc:
        with (
            tc.tile_pool(name="const", bufs=1) as constp,
            tc.tile_pool(name="zbig", bufs=2) as zbig,
            tc.tile_pool(name="sqp", bufs=2) as sqp,
            tc.tile_pool(name="small", bufs=4) as small,
            tc.tile_pool(name="statp", bufs=4) as statp,
            tc.tile_pool(name="epool", bufs=2) as epool,
            tc.tile_pool(name="stgp", bufs=2) as stgp,
            tc.tile_pool(name="ohp", bufs=4) as ohp,
            tc.tile_pool(name="psg", bufs=2, space="PSUM") as psg,
            tc.tile_pool(name="pshi", bufs=1, space="PSUM") as pshi,
            tc.tile_pool(name="psei", bufs=1, space="PSUM") as psei,
        ):
            # ---------------- constants (outside repeat loop) ----------------
            msa_sb = [constp.tile([128, NSEQ], f32, tag=f"msa{j}", name=f"msa_sb{j}")
                      for j in range(JT)]
            for j in range(JT):
                nc.sync.dma_start(out=msa_sb[j], in_=msaT[j * 128:(j + 1) * 128, :])
            wext_sb = constp.tile([DP, C + 1], f32, tag="wext")
            nc.sync.dma_start(out=wext_sb, in_=wext[:, :])
            mask_sb = constp.tile([128, JT * SLAB], f32, tag="mask")
            nc.sync.dma_start(out=mask_sb, in_=maskt[:, :])
            gvec_sb = constp.tile([128, C], f32, tag="gvec")
            nc.sync.dma_start(out=gvec_sb, in_=bcast(gvec[:, :], 128))
            c0p_sb = constp.tile([128, C], f32, tag="c0p")
            nc.sync.dma_start(out=c0p_sb, in_=bcast(c0p[:, :], 128))
            gw1_sb = constp.tile([SLAB, DS], f32, tag="gw1")
            nc.sync.dma_start(out=gw1_sb, in_=bcast(gw1[:, :], SLAB))
            b1r_sb = constp.tile([SLAB, DS], f32, tag="b1r")
            nc.sync.dma_start(out=b1r_sb, in_=bcast(b1r[:, :], SLAB))
            consts_sb = constp.tile([SLAB, 2], f32, tag="consts")
            nc.sync.dma_start(out=consts_sb, in_=bcast(consts[:, :], SLAB))
            siT_sb = [constp.tile([128, SLAB], f32, tag=f"siT{k}", name=f"siT_sb{k}")
                      for k in range(3)]
            W1_sb = [constp.tile([128, DS], f32, tag=f"W1{k}", name=f"W1_sb{k}")
                     for k in range(3)]
            for k in range(3):
                nc.sync.dma_start(out=siT_sb[k], in_=siT[k * 128:(k + 1) * 128, :])
                nc.sync.dma_start(out=W1_sb[k], in_=W1[k * 128:(k + 1) * 128, :])
            ones_col = constp.tile([128, 1], f32, tag="ones")
            nc.vector.memset(ones_col, 1.0)
            eps4_col = constp.tile([128, 1], f32, tag="eps4")
            nc.vector.memset(eps4_col, 4.0 * EPS)
            eps1_col = constp.tile([SLAB, 1], f32, tag="eps1")
            nc.vector.memset(eps1_col, EPS)
            ei_col = constp.tile([SLAB, 1], f32, tag="eicol")

            import contextlib
            loop_cm = tc.For_i(0, repeat, 1) if repeat > 1 else contextlib.nullcontext()
            with loop_cm:
                # ---------------- ei head ----------------
                ps_e = psei.tile([SLAB, DS], f32)
                for k in range(3):
                    nc.tensor.matmul(ps_e, siT_sb[k], W1_sb[k],
                                     start=(k == 0), stop=(k == 2))
                h_sb = small.tile([SLAB, DS], f32, tag="h")
                nc.vector.tensor_add(out=h_sb, in0=ps_e, in1=b1r_sb)
                e_sb = small.tile([SLAB, DS], f32, tag="erf")
                act_fn = ACT_F.Erf if act == "erf" else ACT_F.Tanh
                nc.scalar.activation(out=e_sb, in_=h_sb, func=act_fn,
                                     scale=0.7071067811865476)
                nc.vector.tensor_scalar_add(out=e_sb, in0=e_sb, scalar1=1.0)
                hg_sb = small.tile([SLAB, DS], f32, tag="hg")
                nc.vector.scalar_tensor_tensor(out=hg_sb, in0=h_sb, scalar=0.5,
                                               in1=e_sb, op0=OP.mult, op1=OP.mult)
                stats = small.tile([SLAB, 6], f32, tag="bnst")
                nc.vector.bn_stats(out=stats, in_=hg_sb)
                mv = small.tile([SLAB, 2], f32, tag="bnagg")
                nc.vector.bn_aggr(out=mv, in_=stats)
                tmp = small.tile([SLAB, DS], f32, tag="eitmp")
                nc.vector.tensor_mul(out=tmp, in0=hg_sb, in1=gw1_sb)
                s_col = small.tile([SLAB, 1], f32, tag="scol")
                nc.vector.tensor_reduce(out=s_col, in_=tmp, axis=AX.X, op=OP.add)
                std_e = small.tile([SLAB, 1], f32, tag="stde")
                nc.scalar.activation(out=std_e, in_=mv[:, 1:2], func=ACT_F.Sqrt,
                                     bias=eps1_col, scale=1.0)
                rstd_e = small.tile([SLAB, 1], f32, tag="rstde")
                nc.vector.reciprocal(out=rstd_e, in_=std_e)
                t1 = small.tile([SLAB, 1], f32, tag="t1")
                nc.vector.tensor_mul(out=t1, in0=mv[:, 0:1], in1=consts_sb[:, 0:1])
                t2 = small.tile([SLAB, 1], f32, tag="t2")
                nc.vector.tensor_sub(out=t2, in0=s_col, in1=t1)
                t3 = small.tile([SLAB, 1], f32, tag="t3")
                nc.vector.tensor_mul(out=t3, in0=t2, in1=rstd_e)
                nc.vector.tensor_add(out=ei_col, in0=t3, in1=consts_sb[:, 1:2])
                nc.sync.dma_start(out=out_ei[:, :], in_=ei_col)

                # ---------------- pair head + einsum ----------------
                ps_hi = [pshi.tile([SLAB, NSEQ], f32, tag=f"hi{j}", name=f"ps_hi{j}")
                         for j in range(JT)]
                for jt in range(JT):
                    jsl = slice(jt * 128, (jt + 1) * 128)
                    zr_big = zbig.tile([128, SLAB, 128], f32, tag="zr")
                    zc_big = zbig.tile([128, SLAB, 128], f32, tag="zc")
                    nc.sync.dma_start(
                        out=zr_big, in_=zrT[:, :, jsl].rearrange("i d j -> d i j"))
                    nc.sync.dma_start(
                        out=zc_big, in_=zcT[:, :, jsl].rearrange("i d j -> d i j"))

                    E_jt = epool.tile([128, C, SLAB], edt, tag="E")
                    stage_big = stgp.tile([128, SLAB, C], f32, tag="stage")
                    for g in range(NG):
                        gs = slice(g * G16, (g + 1) * G16)
                        # s = zr + zc (== 2*zs), in place, one group at a time
                        nc.vector.tensor_add(out=zr_big[:, gs, :],
                                             in0=zr_big[:, gs, :],
                                             in1=zc_big[:, gs, :])
                        s2_grp = sqp.tile([128, G16, 128], f32, tag="s2")
                        nc.scalar.activation(out=s2_grp, in_=zr_big[:, gs, :],
                                             func=ACT_F.Square)
                        pg = psg.tile([128, G16 * 24], f32, tag="pg")
                        pg3 = pg.rearrange("p (i k) -> p i k", k=24)
                        for ii in range(G16):
                            i = g * G16 + ii
                            nc.tensor.matmul(pg[:, ii * 24: ii * 24 + 23],
                                             zr_big[:, i, :], wext_sb,
                                             start=True, stop=True)
                            nc.tensor.matmul(pg[:, ii * 24 + 23: ii * 24 + 24],
                                             s2_grp[:, ii, :], ones_col,
                                             start=True, stop=True)
                        # batched stats for the 16 rows
                        mu_g = statp.tile([128, G16, 1], f32, tag="mu")
                        nc.vector.tensor_scalar_mul(out=mu_g, in0=pg3[:, :, 22:23],
                                                    scalar1=1.0 / DP)
                        sqmu = statp.tile([128, G16, 1], f32, tag="sqmu")
                        nc.vector.tensor_mul(out=sqmu, in0=mu_g, in1=mu_g)
                        var_g = statp.tile([128, G16, 1], f32, tag="var")
                        nc.vector.scalar_tensor_tensor(
                            out=var_g, in0=pg3[:, :, 23:24], scalar=1.0 / DP,
                            in1=sqmu, op0=OP.mult, op1=OP.subtract)
                        std_g = statp.tile([128, G16, 1], f32, tag="std")
                        nc.scalar.activation(out=std_g, in_=var_g, func=ACT_F.Sqrt,
                                             bias=eps4_col, scale=1.0)
                        rstd_g = statp.tile([128, G16, 1], f32, tag="rstd")
                        nc.vector.reciprocal(out=rstd_g, in_=std_g)
                        for ii in range(G16):
                            i = g * G16 + ii
                            t_mg = small.tile([128, C], f32, tag="tmg")
                            nc.vector.scalar_tensor_tensor(
                                out=t_mg, in0=gvec_sb, scalar=mu_g[:, ii, :],
                                in1=pg[:, ii * 24: ii * 24 + 22],
                                op0=OP.mult, op1=OP.subtract)
                            eij_w = small.tile([128, C], f32, tag="eijw")
                            nc.vector.scalar_tensor_tensor(
                                out=eij_w, in0=t_mg, scalar=rstd_g[:, ii, :],
                                in1=c0p_sb, op0=OP.mult, op1=OP.subtract)
                            # eij_w == -eij ; mask columns hold {-1, 0}
                            mcol = mask_sb[:, jt * SLAB + i: jt * SLAB + i + 1]
                            nc.gpsimd.tensor_scalar_mul(out=E_jt[:, :, i],
                                                        in0=eij_w, scalar1=mcol)
                            nc.gpsimd.tensor_scalar_mul(out=stage_big[:, i, :],
                                                        in0=eij_w, scalar1=mcol)
                    nc.sync.dma_start(
                        out=out_eij[:, jsl, :].rearrange("i j c -> j i c"),
                        in_=stage_big)

                    # einsum over this j-block: 21 non-gap classes
                    for c in range(C - 1):
                        oh_t = ohp.tile([128, NSEQ], edt, tag="oh")
                        nc.gpsimd.tensor_single_scalar(
                            out=oh_t, in_=msa_sb[jt], scalar=float(c),
                            op=OP.is_equal)
                        nc.tensor.matmul(ps_hi[jt], E_jt[:, c, :], oh_t[:, :],
                                         start=(c == 0), stop=(c == C - 2))

                a_t = small.tile([SLAB, NSEQ], f32, tag="hiacc")
                nc.vector.tensor_scalar(out=a_t, in0=ps_hi[0], scalar1=ei_col,
                                        scalar2=None, op0=OP.add)
                b_t = small.tile([SLAB, NSEQ], f32, tag="hiacc2")
                nc.vector.tensor_add(out=b_t, in0=ps_hi[1], in1=a_t)
                hi_sb = small.tile([SLAB, NSEQ], f32, tag="hisb")
                nc.vector.tensor_add(out=hi_sb, in0=ps_hi[2], in1=b_t)
                nc.sync.dma_start(out=out_hi[:, :], in_=hi_sb)

    nc.compile()
    return nc


def _build_jitted(nc, n_cores):
    """Reusable jitted SPMD executor (mirrors bass2jax.run_bass_via_pjrt)."""
    import jax
    import concourse.mybir as mybir
    from jax.sharding import Mesh, PartitionSpec
    from jax.experimental.shard_map import shard_map
    from concourse.bass2jax import (_bass_exec_p, install_neuronx_cc_hook,
                                    partition_id_tensor)

    install_neuronx_cc_hook()
    partition_name = nc.partition_id_tensor.name if nc.partition_id_tensor else None
    in_names, out_names, out_avals, zero_outs = [], [], [], []
    for alloc in nc.m.functions[0].allocations:
        if not isinstance(alloc, mybir.MemoryLocationSet):
            continue
        name = alloc.memorylocations[0].name
        if alloc.kind == "ExternalInput":
            if name != partition_name:
                in_names.append(name)
        elif alloc.kind == "ExternalOutput":
            out_names.append(name)
            shape = tuple(alloc.tensor_shape)
            dtype = mybir.dt.np(alloc.dtype)
            out_avals.append(jax.core.ShapedArray(shape, dtype))
            zero_outs.append(np.zeros(shape, dtype))
    n_params = len(in_names)
    full_in_names = list(in_names) + list(out_names)
    if partition_name is not None:
        full_in_names.append(partition_name)

    def _body(*args):
        operands = list(args)
        if partition_name is not None:
            operands.append(partition_id_tensor())
        outs = _bass_exec_p.bind(
            *operands,
            out_avals=tuple(out_avals),
            in_names=tuple(full_in_names),
            out_names=tuple(out_names),
            lowering_input_output_aliases=(),
            sim_require_finite=True,
            sim_require_nnan=True,
            nc=nc,
        )
        return tuple(outs)

    devices = jax.devices()[:n_cores]
    mesh = Mesh(np.asarray(devices), ("core",))
    in_specs = (PartitionSpec("core"),) * (n_params + len(out_names))
    out_specs = (PartitionSpec("core"),) * len(out_names)
    sharded = jax.jit(
        shard_map(_body, mesh=mesh, in_specs=in_specs, out_specs=out_specs,
                  check_rep=False),
        keep_unused=True,
    )
    return sharded, mesh, in_names, out_names, zero_outs


def get_executor(repeat=1):
    key = ("exec", repeat)
    if key not in _CACHE:
        nc = _build_program(repeat=repeat)
        _CACHE[key] = _build_jitted(nc, NCORES)
    return _CACHE[key]


def prepare_in_maps(si, zij, msa, W1, b1, ln1_g, ln1_b, W2, b2,
                    lnp_g, lnp_b, Wp, bp):
    si = np.asarray(si, F32)
    zij = np.asarray(zij, F32)
    msa = np.asarray(msa)
    W1 = np.asarray(W1, F32); b1 = np.asarray(b1, F32)
    ln1_g = np.asarray(ln1_g, F32); ln1_b = np.asarray(ln1_b, F32)
    W2 = np.asarray(W2, F32); b2 = np.asarray(b2, F32)
    lnp_g = np.asarray(lnp_g, F32); lnp_b = np.asarray(lnp_b, F32)
    Wp = np.asarray(Wp, F32); bp = np.asarray(bp, F32)

    z = zij[0]
    msaT = np.ascontiguousarray(msa[0].T.astype(F32))            # [N, NSEQ]
    Wg = lnp_g[:, None] * Wp                                     # [DP, C]
    wext = np.ascontiguousarray(
        np.concatenate([Wg, np.ones((DP, 1), F32)], axis=1))     # [DP, C+1]
    gvec = np.ascontiguousarray((lnp_g @ Wp)[None])              # [1, C]
    c0p = np.ascontiguousarray((lnp_b @ Wp + bp)[None])          # [1, C]
    gw1 = np.ascontiguousarray((ln1_g * W2[:, 0])[None])         # [1, DS]
    consts = np.array([[gw1.sum(), float(ln1_b @ W2[:, 0] + b2[0])]], F32)
    b1row = np.ascontiguousarray(b1[None])                       # [1, DS]
    W1c = np.ascontiguousarray(W1)

    in_maps = []
    for d in range(NCORES):
        i0 = d * SLAB
        zr = np.ascontiguousarray(z[i0:i0 + SLAB].transpose(0, 2, 1))
        zc = np.ascontiguousarray(z[:, i0:i0 + SLAB].transpose(1, 2, 0))
        siT = np.ascontiguousarray(si[0, i0:i0 + SLAB].T)
        m = np.full((128, JT * SLAB), -1.0, F32)
        for i in range(SLAB):
            g = i0 + i
            m[g % 128, (g // 128) * SLAB + i] = 0.0
        in_maps.append({
            "zrT": zr, "zcT": zc, "msaT": msaT, "siT": siT, "W1": W1c,
            "wext": wext, "gvec": gvec, "c0p": c0p, "gw1": gw1,
            "b1r": b1row, "consts": consts, "maskt": m,
        })
    return in_maps


def run_spmd(in_maps, repeat=1, device_inputs=None, convert=True):
    """Execute; returns (per_core_results, timing_closure_inputs)."""
    import jax
    from jax.sharding import NamedSharding, PartitionSpec

    sharded, mesh, in_names, out_names, zero_outs = get_executor(repeat)
    if device_inputs is None:
        sh = NamedSharding(mesh, PartitionSpec("core"))
        concat_in = [np.concatenate([m[n] for m in in_maps], axis=0)
                     for n in in_names]
        concat_zero = [np.zeros((NCORES * z.shape[0], *z.shape[1:]), z.dtype)
                       for z in zero_outs]
        device_inputs = ([jax.device_put(a, sh) for a in concat_in],
                         [jax.device_put(a, sh) for a in concat_zero])
    dev_in, dev_zero = device_inputs
    outs = sharded(*dev_in, *dev_zero)
    jax.block_until_ready(outs)
    if not convert:
        return None, device_inputs
    per_core = []
    for c in range(NCORES):
        r = {}
        for idx, n in enumerate(out_names):
            full = np.asarray(outs[idx])
            shp = zero_outs[idx].shape
            r[n] = full.reshape(NCORES, *shp)[c]
        per_core.append(r)
    return per_core, device_inputs


def kernel(si, zij, msa, W1, b1, ln1_g, ln1_b, W2, b2, lnp_g, lnp_b, Wp, bp):
    in_maps = prepare_in_maps(si, zij, msa, W1, b1, ln1_g, ln1_b, W2, b2,
                              lnp_g, lnp_b, Wp, bp)
    results, _ = run_spmd(in_maps, repeat=1)

    logits = np.zeros((B, NSEQ, N, 1), F32)
    eij = np.zeros((B, N, N, C), F32)
    ei = np.zeros((B, N, 1), F32)
    for d in range(NCORES):
        i0 = d * SLAB
        r = results[d]
        eij[0, i0:i0 + SLAB] = r["out_eij"]
        logits[0, :, i0:i0 + SLAB, 0] = r["out_hi"].T
        ei[0, i0:i0 + SLAB, 0] = r["out_ei"][:, 0]
    return logits, eij, ei


# revision 11
# speedup vs baseline: 2.7514x; 2.3789x over previous
"""Trainium2 Bass kernel for CoevolutionDeletionHead (8-core SPMD).

Contract: kernel(**inputs) takes the FULL unsharded inputs (numpy, keyed as in
setup_inputs) and returns the FULL outputs (logits, eij, ei).

Sharding: the i dimension (N_res=384 rows of zij / eij / output) is split into
8 slabs of 48 rows, one per NeuronCore. Each core receives its zij row-slab
and column-slab pre-transposed on the host to [i, d, j] layout so the dp=128
contraction dim lands on SBUF partitions (no on-device transposes). msa and
the small weights are replicated.

Device math (exact algebraic refactor of the reference):
  s = z[i,:,j] + z[j,:,i]            (= 2*zs, [d=128, j] tiles)
  LN(zs) @ Wp + bp  ==  (s@Wg - mean(s)*G) * rsqrt(var(s) + 4*eps) + c0
     with Wg = lnp_g*Wp, G = lnp_g@Wp, c0 = lnp_b@Wp + bp
  (the 0.5 symmetrization scale cancels exactly when eps is scaled by 4)
  row stats come from the same matmul: rhs = [Wg | ones] gives sum(s);
  a second 1-column matmul on s^2 gives sum(s^2).
  hi[i,m] = sum_{c<21, j} eij[i,j,c] * (msa[m,j]==c)  as 63 accumulating
  matmuls (per-class one-hot built on-chip by is_equal against msa^T).
  ei head: gelu_exact via Erf, LN folded into the final dot product.
"""

import numpy as np

# problem dims (hardcoded per contract)
B, N, NSEQ, DS, DP, C = 1, 384, 256, 384, 128, 22
GAP = 21
NCORES = 8
SLAB = N // NCORES          # 48
JT = N // 128               # 3 j-tiles
EPS = 1e-5
F32 = np.float32

_CACHE = {}


def _build_program(repeat=1, act="erf", einsum_f32r=True):
    import concourse.bacc as bacc
    import concourse.bass as bass
    import concourse.mybir as mybir
    import concourse.tile as tile

    f32 = mybir.dt.float32
    f32r = mybir.dt.float32r
    nc = bacc.Bacc("TRN2", target_bir_lowering=False, debug=False)

    # ---- dram tensors (per-core inputs) ----
    zrT = nc.dram_tensor("zrT", [JT, DP, SLAB, 128], f32, kind="ExternalInput")
    zcT = nc.dram_tensor("zcT", [JT, DP, SLAB, 128], f32, kind="ExternalInput")
    msaT = nc.dram_tensor("msaT", [N, NSEQ], f32, kind="ExternalInput")
    siT = nc.dram_tensor("siT", [DS, SLAB], f32, kind="ExternalInput")
    W1 = nc.dram_tensor("W1", [DS, DS], f32, kind="ExternalInput")
    wext = nc.dram_tensor("wext", [DP, C + 1], f32, kind="ExternalInput")
    gvec = nc.dram_tensor("gvec", [1, C], f32, kind="ExternalInput")
    c0p = nc.dram_tensor("c0p", [1, C], f32, kind="ExternalInput")
    gw1 = nc.dram_tensor("gw1", [1, DS], f32, kind="ExternalInput")
    b1r = nc.dram_tensor("b1r", [1, DS], f32, kind="ExternalInput")
    consts = nc.dram_tensor("consts", [1, 2], f32, kind="ExternalInput")

    out_eij = nc.dram_tensor("out_eij", [JT, 128, SLAB, C], f32, kind="ExternalOutput")
    out_hi = nc.dram_tensor("out_hi", [SLAB, NSEQ], f32, kind="ExternalOutput")
    out_ei = nc.dram_tensor("out_ei", [SLAB, 1], f32, kind="ExternalOutput")

    def bcast(ap, p):
        # DMA-source AP replicating a [1, n] dram row across p partitions
        return bass.AP(tensor=ap.tensor, offset=ap.offset,
                       ap=[[0, p]] + list(ap.ap[1:]))

    AX = mybir.AxisListType
    OP = mybir.AluOpType
    ACT_F = mybir.ActivationFunctionType
    G16 = 16                      # i-group size for batched stats
    NG = SLAB // G16              # 3 groups per jt

    edt = f32r if einsum_f32r else f32

    with tile.TileContext(nc) as tc:
        with (
            tc.tile_pool(name="const", bufs=1) as constp,
            tc.tile_pool(name="zbig", bufs=2) as zbig,
            tc.tile_pool(name="sqp", bufs=2) as sqp,
            tc.tile_pool(name="small", bufs=4) as small,
            tc.tile_pool(name="statp", bufs=4) as statp,
            tc.tile_pool(name="epool", bufs=2) as epool,
            tc.tile_pool(name="stgp", bufs=2) as stgp,
            tc.tile_pool(name="ohp", bufs=4) as ohp,
            tc.tile_pool(name="psg", bufs=2, space="PSUM") as psg,
            tc.tile_pool(name="pshi", bufs=1, space="PSUM") as pshi,
            tc.tile_pool(name="psei", bufs=1, space="PSUM") as psei,
        ):
            # ---------------- constants (outside repeat loop) ----------------
            msa_sb = [constp.tile([128, NSEQ], f32, tag=f"msa{j}", name=f"msa_sb{j}")
                      for j in range(JT)]
            for j in range(JT):
                nc.sync.dma_start(out=msa_sb[j], in_=msaT[j * 128:(j + 1) * 128, :])
            wext_sb = constp.tile([DP, C + 1], f32, tag="wext")
            nc.sync.dma_start(out=wext_sb, in_=wext[:, :])
            gvec_sb = constp.tile([128, C], f32, tag="gvec")
            nc.sync.dma_start(out=gvec_sb, in_=bcast(gvec[:, :], 128))
            c0p_sb = constp.tile([128, C], f32, tag="c0p")
            nc.sync.dma_start(out=c0p_sb, in_=bcast(c0p[:, :], 128))
            gw1_sb = constp.tile([SLAB, DS], f32, tag="gw1")
            nc.sync.dma_start(out=gw1_sb, in_=bcast(gw1[:, :], SLAB))
            b1r_sb = constp.tile([SLAB, DS], f32, tag="b1r")
            nc.sync.dma_start(out=b1r_sb, in_=bcast(b1r[:, :], SLAB))
            consts_sb = constp.tile([SLAB, 2], f32, tag="consts")
            nc.sync.dma_start(out=consts_sb, in_=bcast(consts[:, :], SLAB))
            siT_sb = [constp.tile([128, SLAB], f32, tag=f"siT{k}", name=f"siT_sb{k}")
                      for k in range(3)]
            W1_sb = [constp.tile([128, DS], f32, tag=f"W1{k}", name=f"W1_sb{k}")
                     for k in range(3)]
            for k in range(3):
                nc.sync.dma_start(out=siT_sb[k], in_=siT[k * 128:(k + 1) * 128, :])
                nc.sync.dma_start(out=W1_sb[k], in_=W1[k * 128:(k + 1) * 128, :])
            ones_col = constp.tile([128, 1], f32, tag="ones")
            nc.vector.memset(ones_col, 1.0)
            eps4_col = constp.tile([128, 1], f32, tag="eps4")
            nc.vector.memset(eps4_col, 4.0 * EPS)
            eps1_col = constp.tile([SLAB, 1], f32, tag="eps1")
            nc.vector.memset(eps1_col, EPS)
            ei_col = constp.tile([SLAB, 1], f32, tag="eicol")

            import contextlib
            loop_cm = tc.For_i(0, repeat, 1) if repeat > 1 else contextlib.nullcontext()
            with loop_cm:
                # ---------------- ei head ----------------
                ps_e = psei.tile([SLAB, DS], f32)
                for k in range(3):
                    nc.tensor.matmul(ps_e, siT_sb[k], W1_sb[k],
                                     start=(k == 0), stop=(k == 2))
                h_sb = small.tile([SLAB, DS], f32, tag="h")
                nc.vector.tensor_add(out=h_sb, in0=ps_e, in1=b1r_sb)
                e_sb = small.tile([SLAB, DS], f32, tag="erf")
                act_fn = ACT_F.Erf if act == "erf" else ACT_F.Tanh
                nc.scalar.activation(out=e_sb, in_=h_sb, func=act_fn,
                                     scale=0.7071067811865476)
                nc.vector.tensor_scalar_add(out=e_sb, in0=e_sb, scalar1=1.0)
                hg_sb = small.tile([SLAB, DS], f32, tag="hg")
                nc.vector.scalar_tensor_tensor(out=hg_sb, in0=h_sb, scalar=0.5,
                                               in1=e_sb, op0=OP.mult, op1=OP.mult)
                stats = small.tile([SLAB, 6], f32, tag="bnst")
                nc.vector.bn_stats(out=stats, in_=hg_sb)
                mv = small.tile([SLAB, 2], f32, tag="bnagg")
                nc.vector.bn_aggr(out=mv, in_=stats)
                tmp = small.tile([SLAB, DS], f32, tag="eitmp")
                nc.vector.tensor_mul(out=tmp, in0=hg_sb, in1=gw1_sb)
                s_col = small.tile([SLAB, 1], f32, tag="scol")
                nc.vector.tensor_reduce(out=s_col, in_=tmp, axis=AX.X, op=OP.add)
                std_e = small.tile([SLAB, 1], f32, tag="stde")
                nc.scalar.activation(out=std_e, in_=mv[:, 1:2], func=ACT_F.Sqrt,
                                     bias=eps1_col, scale=1.0)
                rstd_e = small.tile([SLAB, 1], f32, tag="rstde")
                nc.vector.reciprocal(out=rstd_e, in_=std_e)
                t1 = small.tile([SLAB, 1], f32, tag="t1")
                nc.vector.tensor_mul(out=t1, in0=mv[:, 0:1], in1=consts_sb[:, 0:1])
                t2 = small.tile([SLAB, 1], f32, tag="t2")
                nc.vector.tensor_sub(out=t2, in0=s_col, in1=t1)
                t3 = small.tile([SLAB, 1], f32, tag="t3")
                nc.vector.tensor_mul(out=t3, in0=t2, in1=rstd_e)
                nc.vector.tensor_add(out=ei_col, in0=t3, in1=consts_sb[:, 1:2])
                nc.sync.dma_start(out=out_ei[:, :], in_=ei_col)

                # ---------------- pair head + einsum ----------------
                ps_hi = [pshi.tile([SLAB, NSEQ], f32, tag=f"hi{j}", name=f"ps_hi{j}")
                         for j in range(JT)]
                for jt in range(JT):
                    zr_big = zbig.tile([128, SLAB, 128], f32, tag="zr")
                    zc_big = zbig.tile([128, SLAB, 128], f32, tag="zc")
                    nc.sync.dma_start(out=zr_big, in_=zrT[jt])
                    nc.sync.dma_start(out=zc_big, in_=zcT[jt])

                    E_jt = epool.tile([128, C, SLAB], edt, tag="E")
                    stage_big = stgp.tile([128, SLAB, C], f32, tag="stage")
                    for g in range(NG):
                        gs = slice(g * G16, (g + 1) * G16)
                        # s = zr + zc (== 2*zs), in place, one group at a time
                        nc.vector.tensor_add(out=zr_big[:, gs, :],
                                             in0=zr_big[:, gs, :],
                                             in1=zc_big[:, gs, :])
                        s2_grp = sqp.tile([128, G16, 128], f32, tag="s2")
                        nc.scalar.activation(out=s2_grp, in_=zr_big[:, gs, :],
                                             func=ACT_F.Square)
                        pg = psg.tile([128, G16 * 24], f32, tag="pg")
                        pg3 = pg.rearrange("p (i k) -> p i k", k=24)
                        for ii in range(G16):
                            i = g * G16 + ii
                            nc.tensor.matmul(pg[:, ii * 24: ii * 24 + 23],
                                             zr_big[:, i, :], wext_sb,
                                             start=True, stop=True)
                            nc.tensor.matmul(pg[:, ii * 24 + 23: ii * 24 + 24],
                                             s2_grp[:, ii, :], ones_col,
                                             start=True, stop=True)
                        # batched stats for the 16 rows
                        mu_g = statp.tile([128, G16, 1], f32, tag="mu")
                        nc.vector.tensor_scalar_mul(out=mu_g, in0=pg3[:, :, 22:23],
                                                    scalar1=1.0 / DP)
                        sqmu = statp.tile([128, G16, 1], f32, tag="sqmu")
                        nc.vector.tensor_mul(out=sqmu, in0=mu_g, in1=mu_g)
                        var_g = statp.tile([128, G16, 1], f32, tag="var")
                        nc.vector.scalar_tensor_tensor(
                            out=var_g, in0=pg3[:, :, 23:24], scalar=1.0 / DP,
                            in1=sqmu, op0=OP.mult, op1=OP.subtract)
                        std_g = statp.tile([128, G16, 1], f32, tag="std")
                        nc.scalar.activation(out=std_g, in_=var_g, func=ACT_F.Sqrt,
                                             bias=eps4_col, scale=1.0)
                        rstd_g = statp.tile([128, G16, 1], f32, tag="rstd")
                        nc.vector.reciprocal(out=rstd_g, in_=std_g)
                        for ii in range(G16):
                            i = g * G16 + ii
                            # host passes wext/gvec with negated Wg/G, so
                            # t_mg = P - mu*G and stage = eij (right sign)
                            t_mg = small.tile([128, C], f32, tag="tmg")
                            nc.vector.scalar_tensor_tensor(
                                out=t_mg, in0=gvec_sb, scalar=mu_g[:, ii, :],
                                in1=pg[:, ii * 24: ii * 24 + 22],
                                op0=OP.mult, op1=OP.subtract)
                            nc.vector.scalar_tensor_tensor(
                                out=stage_big[:, i, :], in0=t_mg,
                                scalar=rstd_g[:, ii, :],
                                in1=c0p_sb, op0=OP.mult, op1=OP.add)
                    nc.vector.tensor_copy(
                        out=E_jt.rearrange("p c i -> p i c"), in_=stage_big)
                    nc.sync.dma_start(out=out_eij[jt], in_=stage_big)

                    # einsum over this j-block: 21 non-gap classes
                    for c in range(C - 1):
                        oh_t = ohp.tile([128, NSEQ], edt, tag="oh")
                        nc.vector.tensor_single_scalar(
                            out=oh_t, in_=msa_sb[jt], scalar=float(c),
                            op=OP.is_equal)
                        nc.tensor.matmul(ps_hi[jt], E_jt[:, c, :], oh_t[:, :],
                                         start=(c == 0), stop=(c == C - 2))

                a_t = small.tile([SLAB, NSEQ], f32, tag="hiacc")
                nc.vector.tensor_scalar(out=a_t, in0=ps_hi[0], scalar1=ei_col,
                                        scalar2=None, op0=OP.add)
                b_t = small.tile([SLAB, NSEQ], f32, tag="hiacc2")
                nc.vector.tensor_add(out=b_t, in0=ps_hi[1], in1=a_t)
                hi_sb = small.tile([SLAB, NSEQ], f32, tag="hisb")
                nc.vector.tensor_add(out=hi_sb, in0=ps_hi[2], in1=b_t)
                nc.sync.dma_start(out=out_hi[:, :], in_=hi_sb)

    nc.compile()
    return nc


def _build_jitted(nc, n_cores):
    """Reusable jitted SPMD executor (mirrors bass2jax.run_bass_via_pjrt)."""
    import jax
    import concourse.mybir as mybir
    from jax.sharding import Mesh, PartitionSpec
    from jax.experimental.shard_map import shard_map
    from concourse.bass2jax import (_bass_exec_p, install_neuronx_cc_hook,
                                    partition_id_tensor)

    install_neuronx_cc_hook()
    partition_name = nc.partition_id_tensor.name if nc.partition_id_tensor else None
    in_names, out_names, out_avals, zero_outs = [], [], [], []
    for alloc in nc.m.functions[0].allocations:
        if not isinstance(alloc, mybir.MemoryLocationSet):
            continue
        name = alloc.memorylocations[0].name
        if alloc.kind == "ExternalInput":
            if name != partition_name:
                in_names.append(name)
        elif alloc.kind == "ExternalOutput":
            out_names.append(name)
            shape = tuple(alloc.tensor_shape)
            dtype = mybir.dt.np(alloc.dtype)
            out_avals.append(jax.core.ShapedArray(shape, dtype))
            zero_outs.append(np.zeros(shape, dtype))
    n_params = len(in_names)
    full_in_names = list(in_names) + list(out_names)
    if partition_name is not None:
        full_in_names.append(partition_name)

    def _body(*args):
        operands = list(args)
        if partition_name is not None:
            operands.append(partition_id_tensor())
        outs = _bass_exec_p.bind(
            *operands,
            out_avals=tuple(out_avals),
            in_names=tuple(full_in_names),
            out_names=tuple(out_names),
            lowering_input_output_aliases=(),
            sim_require_finite=True,
            sim_require_nnan=True,
            nc=nc,
        )
        return tuple(outs)

    devices = jax.devices()[:n_cores]
    mesh = Mesh(np.asarray(devices), ("core",))
    in_specs = (PartitionSpec("core"),) * (n_params + len(out_names))
    out_specs = (PartitionSpec("core"),) * len(out_names)
    sharded = jax.jit(
        shard_map(_body, mesh=mesh, in_specs=in_specs, out_specs=out_specs,
                  check_rep=False),
        keep_unused=True,
    )
    return sharded, mesh, in_names, out_names, zero_outs


def get_executor(repeat=1):
    key = ("exec", repeat)
    if key not in _CACHE:
        nc = _build_program(repeat=repeat)
        _CACHE[key] = _build_jitted(nc, NCORES)
    return _CACHE[key]


def prepare_in_maps(si, zij, msa, W1, b1, ln1_g, ln1_b, W2, b2,
                    lnp_g, lnp_b, Wp, bp):
    si = np.asarray(si, F32)
    zij = np.asarray(zij, F32)
    msa = np.asarray(msa)
    W1 = np.asarray(W1, F32); b1 = np.asarray(b1, F32)
    ln1_g = np.asarray(ln1_g, F32); ln1_b = np.asarray(ln1_b, F32)
    W2 = np.asarray(W2, F32); b2 = np.asarray(b2, F32)
    lnp_g = np.asarray(lnp_g, F32); lnp_b = np.asarray(lnp_b, F32)
    Wp = np.asarray(Wp, F32); bp = np.asarray(bp, F32)

    z = zij[0]
    msaT = np.ascontiguousarray(msa[0].T.astype(F32))            # [N, NSEQ]
    # Wg and G are passed NEGATED: the device computes
    # stage = ((-G)*mu - s@(-Wg))*rstd + c0 = (s@Wg - mu*G)*rstd + c0 = eij
    Wg = lnp_g[:, None] * Wp                                     # [DP, C]
    wext = np.ascontiguousarray(
        np.concatenate([-Wg, np.ones((DP, 1), F32)], axis=1))    # [DP, C+1]
    gvec = np.ascontiguousarray((-(lnp_g @ Wp))[None])           # [1, C]
    c0p = np.ascontiguousarray((lnp_b @ Wp + bp)[None])          # [1, C]
    gw1 = np.ascontiguousarray((ln1_g * W2[:, 0])[None])         # [1, DS]
    consts = np.array([[gw1.sum(), float(ln1_b @ W2[:, 0] + b2[0])]], F32)
    b1row = np.ascontiguousarray(b1[None])                       # [1, DS]
    W1c = np.ascontiguousarray(W1)

    in_maps = []
    for d in range(NCORES):
        i0 = d * SLAB
        # [JT, DP, SLAB, 128]: exactly the SBUF tile layout -> contiguous DMA
        zr = np.ascontiguousarray(
            z[i0:i0 + SLAB].reshape(SLAB, JT, 128, DP).transpose(1, 3, 0, 2))
        zc = np.ascontiguousarray(
            z[:, i0:i0 + SLAB].reshape(JT, 128, SLAB, DP).transpose(0, 3, 2, 1))
        siT = np.ascontiguousarray(si[0, i0:i0 + SLAB].T)
        in_maps.append({
            "zrT": zr, "zcT": zc, "msaT": msaT, "siT": siT, "W1": W1c,
            "wext": wext, "gvec": gvec, "c0p": c0p, "gw1": gw1,
            "b1r": b1row, "consts": consts,
        })
    return in_maps


def run_spmd(in_maps, repeat=1, device_inputs=None, convert=True):
    """Execute; returns (per_core_results, timing_closure_inputs)."""
    import jax
    from jax.sharding import NamedSharding, PartitionSpec

    sharded, mesh, in_names, out_names, zero_outs = get_executor(repeat)
    if device_inputs is None:
        sh = NamedSharding(mesh, PartitionSpec("core"))
        concat_in = [np.concatenate([m[n] for m in in_maps], axis=0)
                     for n in in_names]
        concat_zero = [np.zeros((NCORES * z.shape[0], *z.shape[1:]), z.dtype)
                       for z in zero_outs]
        device_inputs = ([jax.device_put(a, sh) for a in concat_in],
                         [jax.device_put(a, sh) for a in concat_zero])
    dev_in, dev_zero = device_inputs
    outs = sharded(*dev_in, *dev_zero)
    jax.block_until_ready(outs)
    if not convert:
        return None, device_inputs
    per_core = []
    for c in range(NCORES):
        r = {}
        for idx, n in enumerate(out_names):
            full = np.asarray(outs[idx])
            shp = zero_outs[idx].shape
            r[n] = full.reshape(NCORES, *shp)[c]
        per_core.append(r)
    return per_core, device_inputs


def kernel(si, zij, msa, W1, b1, ln1_g, ln1_b, W2, b2, lnp_g, lnp_b, Wp, bp):
    in_maps = prepare_in_maps(si, zij, msa, W1, b1, ln1_g, ln1_b, W2, b2,
                              lnp_g, lnp_b, Wp, bp)
    results, _ = run_spmd(in_maps, repeat=1)

    msa0 = np.asarray(msa)[0]                                  # [NSEQ, N]
    logits = np.zeros((B, NSEQ, N, 1), F32)
    eij = np.zeros((B, N, N, C), F32)
    ei = np.zeros((B, N, 1), F32)
    ii = np.arange(SLAB)
    for d in range(NCORES):
        i0 = d * SLAB
        r = results[d]
        # device eij layout [JT, 128j, SLAB, C] -> [SLAB, N, C]
        e_slab = np.ascontiguousarray(
            r["out_eij"].transpose(2, 0, 1, 3).reshape(SLAB, N, C))
        gi = i0 + ii
        # the device einsum included the j==i diagonal; subtract it on host
        de = e_slab[ii, gi, :]                                 # [SLAB, C]
        mcols = msa0[:, gi]                                    # [NSEQ, SLAB]
        corr = de[ii[None, :], mcols] * (mcols != GAP)         # [NSEQ, SLAB]
        logits[0, :, i0:i0 + SLAB, 0] = r["out_hi"].T - corr
        e_slab[ii, gi, :] = 0.0                                # zero diagonal
        eij[0, i0:i0 + SLAB] = e_slab
        ei[0, i0:i0 + SLAB, 0] = r["out_ei"][:, 0]
    return logits, eij, ei
